# revision 4
# baseline (speedup 1.0000x reference)
"""Trainium2 Bass kernel v5 for per-pixel (untied) local depthwise conv.

Problem: out[n,h,w,c] = sum_{dh,dw} in[n, h+dh-2, w+dw-2, c] * wt[n, h, w, dh*5+dw]
Shapes: in (8,512,512,3) f32, wt (8,512,512,25) f32, 'same' zero padding.

Design (one image per core, 8 cores):
  - dw (column) shifts are baked into per-tap weight planes on the HOST
    (content shifted, zero padded): every DVE operand reads at its natural
    4B-aligned position (keeps tensor_tensor in 2x mode, which is the TRN2
    cap), and the shift reappears as a free column offset on the PE moving
    read, where it costs nothing.  No parity/halo duplication of x beyond
    the 8-row halo.
  - Output in TWO ROW-PASSES (row-pair rp of each partition): psum tile =
    6 banks, bank = (c, rr) = one full 512-wide output row; every
    accumulation matmul is a contiguous 512-elem moving slice.
  - ACT (idle otherwise) replicates each weight chunk across the C=3
    channel positions so the DVE multiply fuses 5 taps per instruction
    ([k5, (r c)6, j516], 3 free dims with merged (r,c)) - fewer DVE ops =
    less per-op overhead on the critical engine.  The last dh group of
    each pass stays as single-tap ops so the tail matmuls overlap.
  - PE accumulates with ONE resident identity stationary (redundant
    LDWEIGHTS deleted from the IR), 300 matmuls of 512 free.
  - Startup DMAs are priority-ordered: first dh-group weights and first
    x rows split across both HWDGE rings before everything else.
"""

import sys

sys.path.insert(0, "/opt/trn_rl_repo")

import numpy as np

import concourse.bass as bass
import concourse.mybir as mybir
from concourse.tile import TileContext
from concourse.bass_utils import run_bass_kernel_spmd

N, H, W, C, K = 8, 512, 512, 3, 5
KK = K * K
N_CORES = 8
RPP = 4                  # output rows per partition
HROWS = RPP + K - 1      # halo rows stored per partition (8)
JW = 520                 # padded x row width (cols -2..517 at j-2)
X_FREE = HROWS * C * JW          # 12480 fp16 elems per partition
PJ = 516                 # weight plane width (union of dw shifts)
WT_TAP = 2 * PJ                  # 1032 elems per (rp, tap) weight chunk
WT_GROUP = K * WT_TAP            # one (rp, dh) group of 5 taps (5160)
WT_FREE = 2 * KK * WT_TAP        # 51600
W3_HALF = K * C * PJ             # c-replicated half-group, one row (7740)
P_TAP = C * 2 * PJ               # 3096 product elems per (rp, tap)
P_HALF = K * C * PJ              # fused half-group products, one row (7740)
O_FREE = C * 2 * W               # 3072 out elems per partition per pass

FUSE = True              # fused 5-tap DVE ops via ACT c-replication


def _dedupe_identity_ldweights(nc):
    """Tile legalization splits every matmul into a standalone InstLdweights
    + non-self-loading InstMatmult.  All our matmuls share one identity
    stationary, so all but the first load per block are redundant: delete
    them and transplant their sync waits/updates onto the next PE
    instruction.  _split_multi_waits legalizes any multi-wait afterwards."""
    n_del = 0
    for f in nc.m.functions:
        for bb in f.blocks:
            seen_sig = None
            pending_waits, pending_updates = [], []
            new_insts = []
            for inst in bb.instructions:
                if isinstance(inst, mybir.InstLdweights):
                    sig = repr(inst.ins[0])
                    if seen_sig == sig:
                        si = inst.sync_info
                        if si is not None:
                            pending_waits.extend(si.on_wait or [])
                            pending_updates.extend(si.on_update or [])
                        n_del += 1
                        continue
                    seen_sig = sig
                elif (pending_waits or pending_updates) and \
                        inst.engine == mybir.EngineType.PE:
                    si = inst.sync_info
                    w = list(si.on_wait) if (si and si.on_wait) else []
                    u = list(si.on_update) if (si and si.on_update) else []
                    inst.sync_info = mybir.SyncInfo(
                        on_wait=pending_waits + w,
                        on_update=pending_updates + u,
                    )
                    pending_waits, pending_updates = [], []
                new_insts.append(inst)
            assert not pending_waits and not pending_updates
            bb.instructions = new_insts
    return n_del


def _split_multi_waits(nc):
    """This walrus build encodes at most ONE sync-wait per instruction;
    hoist extra waits onto single-wait NOPs on the same engine."""
    n_split = 0
    for f in nc.m.functions:
        for bb in f.blocks:
            new_insts = []
            changed = False
            for inst in bb.instructions:
                si = inst.sync_info
                waits = list(si.on_wait) if (si is not None and si.on_wait) else []
                if len(waits) > 1:
                    changed = True
                    for w in waits[:-1]:
                        nop = mybir.InstNoOp(
                            name=nc.get_next_instruction_name(),
                            engine=inst.engine,
                            sync_info=mybir.SyncInfo(on_wait=[w], on_update=[]),
                            bass_nofuse=True,
                        )
                        new_insts.append(nop)
                        n_split += 1
                    inst.sync_info = mybir.SyncInfo(
                        on_wait=[waits[-1]],
                        on_update=list(si.on_update) if si.on_update else [],
                    )
                new_insts.append(inst)
            if changed:
                bb.instructions = new_insts
    return n_split


_NC_CACHE = None


def _build_program():
    global _NC_CACHE
    if _NC_CACHE is not None:
        return _NC_CACHE

    fp16 = mybir.dt.float16
    f32 = mybir.dt.float32

    nc = bass.Bass("TRN2", target_bir_lowering=False, debug=False,
                   num_devices=N_CORES)
    xbuf = nc.dram_tensor("xbuf", [128, X_FREE], fp16, kind="ExternalInput").ap()
    # wtbuf[p, rp, k, rr, j']
    wtbuf = nc.dram_tensor("wtbuf", [128, WT_FREE], fp16,
                           kind="ExternalInput").ap()
    ident = nc.dram_tensor("ident", [128, 128], fp16, kind="ExternalInput").ap()
    out = nc.dram_tensor("out", [2, 128, O_FREE], fp16,
                         kind="ExternalOutput").ap()

    with TileContext(nc) as tc:
        with (
            tc.tile_pool(name="xpool", bufs=1) as xpool,
            tc.tile_pool(name="wtpool", bufs=3) as wtpool,
            tc.tile_pool(name="w3pool", bufs=3) as w3pool,
            tc.tile_pool(name="ppool", bufs=3) as ppool,
            tc.tile_pool(name="spool", bufs=5) as spool,
            tc.tile_pool(name="outpool", bufs=1) as outpool,
            tc.tile_pool(name="psumpool", bufs=1, space="PSUM") as psumpool,
        ):
            id_t = xpool.tile([128, 128], fp16)
            x_t = xpool.tile([128, X_FREE], fp16)
            XR = C * JW

            # --- priority-ordered input DMAs -------------------------------
            # ring A (sync): x rows 0-1 (first group's rows), then the rest
            # ring B (scalar): weight group (rp0, dh0) first
            wt_tiles = {}

            def wdma(eng, u):
                rp, dh = u // K, u % K
                t = wtpool.tile([128, WT_GROUP], fp16, tag="wt",
                                name=f"wt_{rp}_{dh}")
                eng.dma_start(out=t[:],
                              in_=wtbuf[:, u * WT_GROUP:(u + 1) * WT_GROUP])
                wt_tiles[(rp, dh)] = t

            nc.sync.dma_start(out=x_t[:, 0:2 * XR], in_=xbuf[:, 0:2 * XR])
            wdma(nc.scalar, 0)                       # (rp0, dh0)
            nc.sync.dma_start(out=x_t[:, 2 * XR:4 * XR],
                              in_=xbuf[:, 2 * XR:4 * XR])
            nc.sync.dma_start(out=id_t[:], in_=ident[:])
            wdma(nc.sync, 1)                         # (rp0, dh1)
            nc.scalar.dma_start(out=x_t[:, 4 * XR:6 * XR],
                                in_=xbuf[:, 4 * XR:6 * XR])
            wdma(nc.scalar, 2)
            nc.sync.dma_start(out=x_t[:, 6 * XR:], in_=xbuf[:, 6 * XR:])
            for u in range(3, 10):
                wdma(nc.sync if u % 2 else nc.scalar, u)

            xv = x_t[:].rearrange("p (r c j) -> p r c j", r=HROWS, c=C)

            def fused_half(rp, dh, rr):
                """One DVE op for the 5 dw taps of output row (rp, dh, rr):
                [k5, c3, j516].  ACT first replicates the raw weights
                across c (5 small copies)."""
                w3 = w3pool.tile([128, W3_HALF], fp16, tag="w3",
                                 name=f"w3_{rp}_{dh}_{rr}")
                raw = wt_tiles[(rp, dh)][:].rearrange(
                    "p (k r j) -> p k r j", k=K, r=2)
                w3v = w3.rearrange("p (k c j) -> p k c j", k=K, c=C)
                for k in range(K):
                    s = raw[:, k, rr][:, None, :].broadcast_to([128, C, PJ])
                    nc.scalar.copy(out=w3v[:, k], in_=s)
                p_t = ppool.tile([128, P_HALF], fp16, tag="p",
                                 name=f"p_{rp}_{dh}_{rr}")
                pv = p_t[:].rearrange("p (k c j) -> p k c j", k=K, c=C)
                r0 = dh + 2 * rp + rr
                xs = xv[:, r0, :, 0:PJ][:, None, :, :].broadcast_to(
                    [128, K, C, PJ])
                nc.vector.tensor_mul(out=pv, in0=xs, in1=w3v)
                return pv

            def single_units(rp, dh):
                """5 single-tap DVE ops for (rp, dh) (used for the last
                group so tail matmuls interleave with production)."""
                raw = wt_tiles[(rp, dh)][:].rearrange(
                    "p (k r j) -> p k r j", k=K, r=2)
                pvs = []
                r0 = dh + 2 * rp
                xs = xv[:, r0:r0 + 2, :, 0:PJ].transpose([0, 2, 1, 3])
                for dw in range(K):
                    p_t = spool.tile([128, P_TAP], fp16, tag="s",
                                     name=f"s_{rp}_{dh}_{dw}")
                    pv = p_t[:].rearrange("p (c r j) -> p c r j", c=C, r=2)
                    wk = raw[:, dw][:, None, :, :].broadcast_to([128, C, 2, PJ])
                    nc.vector.tensor_mul(out=pv, in0=xs, in1=wk)
                    pvs.append(pv)
                return pvs

            for rp in range(2):
                ps_t = psumpool.tile([128, 6 * 512], f32, tag="ps",
                                     name=f"ps_{rp}")
                for di, dh in enumerate(range(K)):
                    first_dh = di == 0
                    last_dh = di == K - 1
                    if FUSE and not last_dh:
                        halves = [fused_half(rp, dh, rr) for rr in range(2)]
                        # mov(dw, c, rr) = halves[rr][:, dw, c, dw:dw+W]
                        movf = lambda dw, c, rr: halves[rr][:, dw, c,
                                                            dw:dw + W]
                    else:
                        singles = single_units(rp, dh)   # [p, c, r, j] each
                        movf = lambda dw, c, rr: singles[dw][:, c, rr,
                                                             dw:dw + W]
                    for dw in range(K):
                        for c in range(C):
                            for rr in range(2):
                                bank = c * 2 + rr
                                nc.tensor.matmul(
                                    ps_t[:, bank * 512:(bank + 1) * 512],
                                    id_t[:],
                                    movf(dw, c, rr),
                                    start=(first_dh and dw == 0),
                                    stop=(last_dh and dw == K - 1),
                                )

                # evacuate: ACT takes banks 0-3, DVE (idle now) banks 4-5
                o_t = outpool.tile([128, O_FREE], fp16)
                nc.scalar.copy(out=o_t[:, 0:4 * 512], in_=ps_t[:, 0:4 * 512])
                nc.vector.tensor_copy(out=o_t[:, 4 * 512:],
                                      in_=ps_t[:, 4 * 512:])
                nc.sync.dma_start(out=out[rp], in_=o_t[:])

    _dedupe_identity_ldweights(nc)
    _split_multi_waits(nc)
    _NC_CACHE = nc
    return nc


def _pack_inputs(input_data: np.ndarray, weights: np.ndarray):
    """Host-side layout + fp16 conversion into per-core SBUF-ready buffers."""
    xh = input_data.astype(np.float16)     # (N, H, W, C)
    wh = weights.astype(np.float16)        # (N, H, W, KK)

    identity = np.eye(128, dtype=np.float16)
    in_maps = []
    for n in range(N_CORES):
        # x: [p, rr, c, j]: image row 4p+rr-2, col j-2 (zero pad)
        canvas = np.zeros((C, H + 4, JW), dtype=np.float16)
        canvas[:, 2:2 + H, 2:2 + W] = xh[n].transpose(2, 0, 1)
        sw = np.lib.stride_tricks.sliding_window_view(canvas, HROWS, axis=1)
        sw = sw[:, ::RPP][:, :128]            # (C, 128, JW, 8)
        X = np.ascontiguousarray(
            sw.transpose(1, 3, 0, 2).reshape(128, X_FREE))

        # weight chunks: wtbuf[p, rp, k, rr, j'] = wt[4p+2rp+rr, j'-dw, k]
        wtpad = np.zeros((H, PJ, KK), dtype=np.float16)
        for dw in range(K):
            wtpad[:, dw:dw + W, dw::K] = wh[n][:, :, dw::K]
        WT = np.ascontiguousarray(
            wtpad.reshape(128, 2, 2, PJ, KK)       # p, rp, rr, j', k
            .transpose(0, 1, 4, 2, 3)              # p, rp, k, rr, j'
            .reshape(128, WT_FREE))
        in_maps.append({"xbuf": X, "wtbuf": WT, "ident": identity})
    return in_maps


def _unpack_outputs(results) -> np.ndarray:
    out = np.empty((N, H, W, C), dtype=np.float32)
    for n in range(N_CORES):
        o = results[n]["out"].astype(np.float32)   # (2, 128, O_FREE)
        o = o.reshape(2, 128, C, 2, W)             # rp, p, c, rr, w
        # h = 4p + 2rp + rr
        out[n] = o.transpose(1, 0, 3, 4, 2).reshape(H, W, C)
    return out


def kernel(input_data: np.ndarray, weights: np.ndarray) -> np.ndarray:
    input_data = np.asarray(input_data, dtype=np.float32)
    weights = np.asarray(weights, dtype=np.float32)
    nc = _build_program()
    in_maps = _pack_inputs(input_data, weights)
    res = run_bass_kernel_spmd(nc, in_maps, list(range(N_CORES)))
    return _unpack_outputs(res.results)


if __name__ == "__main__":
    rng = np.random.default_rng(0)
    x = rng.standard_normal((N, H, W, C), dtype=np.float32)
    w = rng.standard_normal((N, H, W, KK), dtype=np.float32) * 0.1
    out = kernel(input_data=x, weights=w)

    xp = np.pad(x, ((0, 0), (2, 2), (2, 2), (0, 0)))
    exp = np.zeros_like(x)
    for k in range(KK):
        dh, dw = k // K, k % K
        exp += xp[:, dh:dh + H, dw:dw + W, :] * w[..., k:k + 1]
    diff = np.linalg.norm(out - exp) / np.linalg.norm(exp)
    print("out", out.shape, out.dtype, "rel err", diff)


# revision 5
# speedup vs baseline: 1.5888x; 1.5888x over previous
"""Trainium2 Bass kernel v5 for per-pixel (untied) local depthwise conv.

Problem: out[n,h,w,c] = sum_{dh,dw} in[n, h+dh-2, w+dw-2, c] * wt[n, h, w, dh*5+dw]
Shapes: in (8,512,512,3) f32, wt (8,512,512,25) f32, 'same' zero padding.

Design (one image per core, 8 cores):
  - dw (column) shifts are baked into per-tap weight planes on the HOST
    (content shifted, zero padded): every DVE operand reads at its natural
    4B-aligned position (keeps tensor_tensor in 2x mode, which is the TRN2
    cap), and the shift reappears as a free column offset on the PE moving
    read, where it costs nothing.  No parity/halo duplication of x beyond
    the 8-row halo.
  - Output in TWO ROW-PASSES (row-pair rp of each partition): psum tile =
    6 banks, bank = (c, rr) = one full 512-wide output row; every
    accumulation matmul is a contiguous 512-elem moving slice.
  - ACT (idle otherwise) replicates each weight chunk across the C=3
    channel positions so the DVE multiply fuses 5 taps per instruction
    ([k5, (r c)6, j516], 3 free dims with merged (r,c)) - fewer DVE ops =
    less per-op overhead on the critical engine.  The last dh group of
    each pass stays as single-tap ops so the tail matmuls overlap.
  - PE accumulates with ONE resident identity stationary (redundant
    LDWEIGHTS deleted from the IR), 300 matmuls of 512 free.
  - Startup DMAs are priority-ordered: first dh-group weights and first
    x rows split across both HWDGE rings before everything else.
"""

import sys

sys.path.insert(0, "/opt/trn_rl_repo")

import numpy as np

import concourse.bass as bass
import concourse.mybir as mybir
from concourse.tile import TileContext
from concourse.bass_utils import run_bass_kernel_spmd

N, H, W, C, K = 8, 512, 512, 3, 5
KK = K * K
N_CORES = 8
RPP = 4                  # output rows per partition
HROWS = RPP + K - 1      # halo rows stored per partition (8)
JW = 520                 # padded x row width (cols -2..517 at j-2)
X_FREE = HROWS * C * JW          # 12480 fp16 elems per partition
PJ = 516                 # weight plane width (union of dw shifts)
WT_TAP = 2 * PJ                  # 1032 elems per (rp, tap) weight chunk
WT_GROUP = K * WT_TAP            # one (rp, dh) group of 5 taps (5160)
WT_FREE = 2 * KK * WT_TAP        # 51600
W3_HALF = K * C * PJ             # c-replicated half-group, one row (7740)
P_TAP = C * 2 * PJ               # 3096 product elems per (rp, tap)
P_HALF = K * C * PJ              # fused half-group products, one row (7740)
O_FREE = C * 2 * W               # 3072 out elems per partition per pass

FUSE = False             # ACT c-replication copies run 1x: too slow


def _dedupe_identity_ldweights(nc):
    """Tile legalization splits every matmul into a standalone InstLdweights
    + non-self-loading InstMatmult.  All our matmuls share one identity
    stationary, so all but the first load per block are redundant: delete
    them and transplant their sync waits/updates onto the next PE
    instruction.  _split_multi_waits legalizes any multi-wait afterwards."""
    n_del = 0
    for f in nc.m.functions:
        for bb in f.blocks:
            seen_sig = None
            pending_waits, pending_updates = [], []
            new_insts = []
            for inst in bb.instructions:
                if isinstance(inst, mybir.InstLdweights):
                    sig = repr(inst.ins[0])
                    if seen_sig == sig:
                        si = inst.sync_info
                        if si is not None:
                            pending_waits.extend(si.on_wait or [])
                            pending_updates.extend(si.on_update or [])
                        n_del += 1
                        continue
                    seen_sig = sig
                elif (pending_waits or pending_updates) and \
                        inst.engine == mybir.EngineType.PE:
                    si = inst.sync_info
                    w = list(si.on_wait) if (si and si.on_wait) else []
                    u = list(si.on_update) if (si and si.on_update) else []
                    inst.sync_info = mybir.SyncInfo(
                        on_wait=pending_waits + w,
                        on_update=pending_updates + u,
                    )
                    pending_waits, pending_updates = [], []
                new_insts.append(inst)
            assert not pending_waits and not pending_updates
            bb.instructions = new_insts
    return n_del


def _split_multi_waits(nc):
    """This walrus build encodes at most ONE sync-wait per instruction;
    hoist extra waits onto single-wait NOPs on the same engine."""
    n_split = 0
    for f in nc.m.functions:
        for bb in f.blocks:
            new_insts = []
            changed = False
            for inst in bb.instructions:
                si = inst.sync_info
                waits = list(si.on_wait) if (si is not None and si.on_wait) else []
                if len(waits) > 1:
                    changed = True
                    for w in waits[:-1]:
                        nop = mybir.InstNoOp(
                            name=nc.get_next_instruction_name(),
                            engine=inst.engine,
                            sync_info=mybir.SyncInfo(on_wait=[w], on_update=[]),
                            bass_nofuse=True,
                        )
                        new_insts.append(nop)
                        n_split += 1
                    inst.sync_info = mybir.SyncInfo(
                        on_wait=[waits[-1]],
                        on_update=list(si.on_update) if si.on_update else [],
                    )
                new_insts.append(inst)
            if changed:
                bb.instructions = new_insts
    return n_split


_NC_CACHE = None


def _build_program():
    global _NC_CACHE
    if _NC_CACHE is not None:
        return _NC_CACHE

    fp16 = mybir.dt.float16
    f32 = mybir.dt.float32

    nc = bass.Bass("TRN2", target_bir_lowering=False, debug=False,
                   num_devices=N_CORES)
    xbuf = nc.dram_tensor("xbuf", [128, X_FREE], fp16, kind="ExternalInput").ap()
    # wtbuf[p, rp, k, rr, j']
    wtbuf = nc.dram_tensor("wtbuf", [128, WT_FREE], fp16,
                           kind="ExternalInput").ap()
    ident = nc.dram_tensor("ident", [128, 128], fp16, kind="ExternalInput").ap()
    out = nc.dram_tensor("out", [2, 128, O_FREE], fp16,
                         kind="ExternalOutput").ap()

    with TileContext(nc) as tc:
        with (
            tc.tile_pool(name="xpool", bufs=1) as xpool,
            tc.tile_pool(name="wtpool", bufs=3) as wtpool,
            tc.tile_pool(name="w3pool", bufs=3) as w3pool,
            tc.tile_pool(name="ppool", bufs=3) as ppool,
            tc.tile_pool(name="spool", bufs=5) as spool,
            tc.tile_pool(name="outpool", bufs=1) as outpool,
            tc.tile_pool(name="psumpool", bufs=1, space="PSUM") as psumpool,
        ):
            id_t = xpool.tile([128, 128], fp16)
            x_t = xpool.tile([128, X_FREE], fp16)
            XR = C * JW

            # --- priority-ordered input DMAs -------------------------------
            # ring A (sync): x rows 0-1 (first group's rows), then the rest
            # ring B (scalar): weight group (rp0, dh0) first
            wt_tiles = {}

            def wdma(eng, u, split_first=False):
                rp, dh = u // K, u % K
                t = wtpool.tile([128, WT_GROUP], fp16, tag="wt",
                                name=f"wt_{rp}_{dh}")
                base = u * WT_GROUP
                if split_first:
                    # first tap lands alone so compute starts immediately
                    eng.dma_start(out=t[:, 0:WT_TAP],
                                  in_=wtbuf[:, base:base + WT_TAP])
                    eng.dma_start(out=t[:, WT_TAP:],
                                  in_=wtbuf[:, base + WT_TAP:base + WT_GROUP])
                else:
                    eng.dma_start(out=t[:],
                                  in_=wtbuf[:, base:base + WT_GROUP])
                wt_tiles[(rp, dh)] = t

            # critical prefix: first tap + first x row-pair, one per ring
            wdma(nc.scalar, 0, split_first=True)     # (rp0, dh0)
            nc.sync.dma_start(out=x_t[:, 0:2 * XR], in_=xbuf[:, 0:2 * XR])
            nc.sync.dma_start(out=id_t[:], in_=ident[:])
            nc.sync.dma_start(out=x_t[:, 2 * XR:4 * XR],
                              in_=xbuf[:, 2 * XR:4 * XR])
            wdma(nc.sync, 1)                         # (rp0, dh1)
            nc.scalar.dma_start(out=x_t[:, 4 * XR:6 * XR],
                                in_=xbuf[:, 4 * XR:6 * XR])
            wdma(nc.scalar, 2)
            nc.sync.dma_start(out=x_t[:, 6 * XR:], in_=xbuf[:, 6 * XR:])
            for u in range(3, 10):
                wdma(nc.sync if u % 2 else nc.scalar, u)

            xv = x_t[:].rearrange("p (r c j) -> p r c j", r=HROWS, c=C)

            def fused_half(rp, dh, rr):
                """One DVE op for the 5 dw taps of output row (rp, dh, rr):
                [k5, c3, j516].  ACT first replicates the raw weights
                across c (5 small copies)."""
                w3 = w3pool.tile([128, W3_HALF], fp16, tag="w3",
                                 name=f"w3_{rp}_{dh}_{rr}")
                raw = wt_tiles[(rp, dh)][:].rearrange(
                    "p (k r j) -> p k r j", k=K, r=2)
                w3v = w3.rearrange("p (k c j) -> p k c j", k=K, c=C)
                for k in range(K):
                    s = raw[:, k, rr][:, None, :].broadcast_to([128, C, PJ])
                    nc.scalar.copy(out=w3v[:, k], in_=s)
                p_t = ppool.tile([128, P_HALF], fp16, tag="p",
                                 name=f"p_{rp}_{dh}_{rr}")
                pv = p_t[:].rearrange("p (k c j) -> p k c j", k=K, c=C)
                r0 = dh + 2 * rp + rr
                xs = xv[:, r0, :, 0:PJ][:, None, :, :].broadcast_to(
                    [128, K, C, PJ])
                nc.vector.tensor_mul(out=pv, in0=xs, in1=w3v)
                return pv

            def single_units(rp, dh):
                """5 single-tap DVE ops for (rp, dh) (used for the last
                group so tail matmuls interleave with production)."""
                raw = wt_tiles[(rp, dh)][:].rearrange(
                    "p (k r j) -> p k r j", k=K, r=2)
                pvs = []
                r0 = dh + 2 * rp
                xs = xv[:, r0:r0 + 2, :, 0:PJ].transpose([0, 2, 1, 3])
                for dw in range(K):
                    p_t = spool.tile([128, P_TAP], fp16, tag="s",
                                     name=f"s_{rp}_{dh}_{dw}")
                    pv = p_t[:].rearrange("p (c r j) -> p c r j", c=C, r=2)
                    wk = raw[:, dw][:, None, :, :].broadcast_to([128, C, 2, PJ])
                    nc.vector.tensor_mul(out=pv, in0=xs, in1=wk)
                    pvs.append(pv)
                return pvs

            for rp in range(2):
                ps_t = psumpool.tile([128, 6 * 512], f32, tag="ps",
                                     name=f"ps_{rp}")
                for di, dh in enumerate(range(K)):
                    first_dh = di == 0
                    last_dh = di == K - 1
                    if FUSE and not last_dh:
                        halves = [fused_half(rp, dh, rr) for rr in range(2)]
                        # mov(dw, c, rr) = halves[rr][:, dw, c, dw:dw+W]
                        movf = lambda dw, c, rr: halves[rr][:, dw, c,
                                                            dw:dw + W]
                    else:
                        singles = single_units(rp, dh)   # [p, c, r, j] each
                        movf = lambda dw, c, rr: singles[dw][:, c, rr,
                                                             dw:dw + W]
                    for dw in range(K):
                        for c in range(C):
                            for rr in range(2):
                                bank = c * 2 + rr
                                nc.tensor.matmul(
                                    ps_t[:, bank * 512:(bank + 1) * 512],
                                    id_t[:],
                                    movf(dw, c, rr),
                                    start=(first_dh and dw == 0),
                                    stop=(last_dh and dw == K - 1),
                                )

                # evacuate: ACT takes banks 0-3, DVE (idle now) banks 4-5;
                # ship each piece as soon as it lands
                o_t = outpool.tile([128, O_FREE], fp16)
                nc.scalar.copy(out=o_t[:, 0:4 * 512], in_=ps_t[:, 0:4 * 512])
                nc.vector.tensor_copy(out=o_t[:, 4 * 512:],
                                      in_=ps_t[:, 4 * 512:])
                nc.sync.dma_start(out=out[rp, :, 0:4 * 512],
                                  in_=o_t[:, 0:4 * 512])
                nc.sync.dma_start(out=out[rp, :, 4 * 512:],
                                  in_=o_t[:, 4 * 512:])

    _dedupe_identity_ldweights(nc)
    _split_multi_waits(nc)
    _NC_CACHE = nc
    return nc


def _pack_inputs(input_data: np.ndarray, weights: np.ndarray):
    """Host-side layout + fp16 conversion into per-core SBUF-ready buffers."""
    xh = input_data.astype(np.float16)     # (N, H, W, C)
    wh = weights.astype(np.float16)        # (N, H, W, KK)

    identity = np.eye(128, dtype=np.float16)
    in_maps = []
    for n in range(N_CORES):
        # x: [p, rr, c, j]: image row 4p+rr-2, col j-2 (zero pad)
        canvas = np.zeros((C, H + 4, JW), dtype=np.float16)
        canvas[:, 2:2 + H, 2:2 + W] = xh[n].transpose(2, 0, 1)
        sw = np.lib.stride_tricks.sliding_window_view(canvas, HROWS, axis=1)
        sw = sw[:, ::RPP][:, :128]            # (C, 128, JW, 8)
        X = np.ascontiguousarray(
            sw.transpose(1, 3, 0, 2).reshape(128, X_FREE))

        # weight chunks: wtbuf[p, rp, k, rr, j'] = wt[4p+2rp+rr, j'-dw, k]
        wtpad = np.zeros((H, PJ, KK), dtype=np.float16)
        for dw in range(K):
            wtpad[:, dw:dw + W, dw::K] = wh[n][:, :, dw::K]
        WT = np.ascontiguousarray(
            wtpad.reshape(128, 2, 2, PJ, KK)       # p, rp, rr, j', k
            .transpose(0, 1, 4, 2, 3)              # p, rp, k, rr, j'
            .reshape(128, WT_FREE))
        in_maps.append({"xbuf": X, "wtbuf": WT, "ident": identity})
    return in_maps


def _unpack_outputs(results) -> np.ndarray:
    out = np.empty((N, H, W, C), dtype=np.float32)
    for n in range(N_CORES):
        o = results[n]["out"].astype(np.float32)   # (2, 128, O_FREE)
        o = o.reshape(2, 128, C, 2, W)             # rp, p, c, rr, w
        # h = 4p + 2rp + rr
        out[n] = o.transpose(1, 0, 3, 4, 2).reshape(H, W, C)
    return out


def kernel(input_data: np.ndarray, weights: np.ndarray) -> np.ndarray:
    input_data = np.asarray(input_data, dtype=np.float32)
    weights = np.asarray(weights, dtype=np.float32)
    nc = _build_program()
    in_maps = _pack_inputs(input_data, weights)
    res = run_bass_kernel_spmd(nc, in_maps, list(range(N_CORES)))
    return _unpack_outputs(res.results)


if __name__ == "__main__":
    rng = np.random.default_rng(0)
    x = rng.standard_normal((N, H, W, C), dtype=np.float32)
    w = rng.standard_normal((N, H, W, KK), dtype=np.float32) * 0.1
    out = kernel(input_data=x, weights=w)

    xp = np.pad(x, ((0, 0), (2, 2), (2, 2), (0, 0)))
    exp = np.zeros_like(x)
    for k in range(KK):
        dh, dw = k // K, k % K
        exp += xp[:, dh:dh + H, dw:dw + W, :] * w[..., k:k + 1]
    diff = np.linalg.norm(out - exp) / np.linalg.norm(exp)
    print("out", out.shape, out.dtype, "rel err", diff)


# revision 6
# speedup vs baseline: 1.5927x; 1.0024x over previous
"""Trainium2 Bass kernel v5 for per-pixel (untied) local depthwise conv.

Problem: out[n,h,w,c] = sum_{dh,dw} in[n, h+dh-2, w+dw-2, c] * wt[n, h, w, dh*5+dw]
Shapes: in (8,512,512,3) f32, wt (8,512,512,25) f32, 'same' zero padding.

Design (one image per core, 8 cores):
  - dw (column) shifts are baked into per-tap weight planes on the HOST
    (content shifted, zero padded): every DVE operand reads at its natural
    4B-aligned position (keeps tensor_tensor in 2x mode, which is the TRN2
    cap), and the shift reappears as a free column offset on the PE moving
    read, where it costs nothing.  No parity/halo duplication of x beyond
    the 8-row halo.
  - Output in TWO ROW-PASSES (row-pair rp of each partition): psum tile =
    6 banks, bank = (c, rr) = one full 512-wide output row; every
    accumulation matmul is a contiguous 512-elem moving slice.
  - ACT (idle otherwise) replicates each weight chunk across the C=3
    channel positions so the DVE multiply fuses 5 taps per instruction
    ([k5, (r c)6, j516], 3 free dims with merged (r,c)) - fewer DVE ops =
    less per-op overhead on the critical engine.  The last dh group of
    each pass stays as single-tap ops so the tail matmuls overlap.
  - PE accumulates with ONE resident identity stationary (redundant
    LDWEIGHTS deleted from the IR), 300 matmuls of 512 free.
  - Startup DMAs are priority-ordered: first dh-group weights and first
    x rows split across both HWDGE rings before everything else.
"""

import sys

sys.path.insert(0, "/opt/trn_rl_repo")

import numpy as np

import concourse.bass as bass
import concourse.mybir as mybir
from concourse.tile import TileContext
from concourse.bass_utils import run_bass_kernel_spmd

N, H, W, C, K = 8, 512, 512, 3, 5
KK = K * K
N_CORES = 8
RPP = 4                  # output rows per partition
HROWS = RPP + K - 1      # halo rows stored per partition (8)
JW = 520                 # padded x row width (cols -2..517 at j-2)
X_FREE = HROWS * C * JW          # 12480 fp16 elems per partition
PJ = 516                 # weight plane width (union of dw shifts)
WT_TAP = 2 * PJ                  # 1032 elems per (rp, tap) weight chunk
WT_GROUP = K * WT_TAP            # one (rp, dh) group of 5 taps (5160)
WT_FREE = 2 * KK * WT_TAP        # 51600
W3_HALF = K * C * PJ             # c-replicated half-group, one row (7740)
P_TAP = C * 2 * PJ               # 3096 product elems per (rp, tap)
P_HALF = K * C * PJ              # fused half-group products, one row (7740)
O_FREE = C * 2 * W               # 3072 out elems per partition per pass

FUSE = False             # ACT c-replication copies run 1x: too slow


def _dedupe_identity_ldweights(nc):
    """Tile legalization splits every matmul into a standalone InstLdweights
    + non-self-loading InstMatmult.  All our matmuls share one identity
    stationary, so all but the first load per block are redundant: delete
    them and transplant their sync waits/updates onto the next PE
    instruction.  _split_multi_waits legalizes any multi-wait afterwards."""
    n_del = 0
    for f in nc.m.functions:
        for bb in f.blocks:
            seen_sig = None
            pending_waits, pending_updates = [], []
            new_insts = []
            for inst in bb.instructions:
                if isinstance(inst, mybir.InstLdweights):
                    sig = repr(inst.ins[0])
                    if seen_sig == sig:
                        si = inst.sync_info
                        if si is not None:
                            pending_waits.extend(si.on_wait or [])
                            pending_updates.extend(si.on_update or [])
                        n_del += 1
                        continue
                    seen_sig = sig
                elif (pending_waits or pending_updates) and \
                        inst.engine == mybir.EngineType.PE:
                    si = inst.sync_info
                    w = list(si.on_wait) if (si and si.on_wait) else []
                    u = list(si.on_update) if (si and si.on_update) else []
                    inst.sync_info = mybir.SyncInfo(
                        on_wait=pending_waits + w,
                        on_update=pending_updates + u,
                    )
                    pending_waits, pending_updates = [], []
                new_insts.append(inst)
            assert not pending_waits and not pending_updates
            bb.instructions = new_insts
    return n_del


def _split_multi_waits(nc):
    """This walrus build encodes at most ONE sync-wait per instruction;
    hoist extra waits onto single-wait NOPs on the same engine."""
    n_split = 0
    for f in nc.m.functions:
        for bb in f.blocks:
            new_insts = []
            changed = False
            for inst in bb.instructions:
                si = inst.sync_info
                waits = list(si.on_wait) if (si is not None and si.on_wait) else []
                if len(waits) > 1:
                    changed = True
                    for w in waits[:-1]:
                        nop = mybir.InstNoOp(
                            name=nc.get_next_instruction_name(),
                            engine=inst.engine,
                            sync_info=mybir.SyncInfo(on_wait=[w], on_update=[]),
                            bass_nofuse=True,
                        )
                        new_insts.append(nop)
                        n_split += 1
                    inst.sync_info = mybir.SyncInfo(
                        on_wait=[waits[-1]],
                        on_update=list(si.on_update) if si.on_update else [],
                    )
                new_insts.append(inst)
            if changed:
                bb.instructions = new_insts
    return n_split


_NC_CACHE = None


def _build_program():
    global _NC_CACHE
    if _NC_CACHE is not None:
        return _NC_CACHE

    fp16 = mybir.dt.float16
    f32 = mybir.dt.float32

    nc = bass.Bass("TRN2", target_bir_lowering=False, debug=False,
                   num_devices=N_CORES)
    xbuf = nc.dram_tensor("xbuf", [128, X_FREE], fp16, kind="ExternalInput").ap()
    # wtbuf[p, rp, k, rr, j']
    wtbuf = nc.dram_tensor("wtbuf", [128, WT_FREE], fp16,
                           kind="ExternalInput").ap()
    ident = nc.dram_tensor("ident", [128, 128], fp16, kind="ExternalInput").ap()
    out = nc.dram_tensor("out", [2, 128, O_FREE], fp16,
                         kind="ExternalOutput").ap()

    with TileContext(nc) as tc:
        with (
            tc.tile_pool(name="xpool", bufs=1) as xpool,
            tc.tile_pool(name="wtpool", bufs=3) as wtpool,
            tc.tile_pool(name="w3pool", bufs=3) as w3pool,
            tc.tile_pool(name="ppool", bufs=3) as ppool,
            tc.tile_pool(name="spool", bufs=16) as spool,
            tc.tile_pool(name="outpool", bufs=1) as outpool,
            tc.tile_pool(name="psumpool", bufs=1, space="PSUM") as psumpool,
        ):
            id_t = xpool.tile([128, 128], fp16)
            x_t = xpool.tile([128, X_FREE], fp16)
            XR = C * JW

            # --- priority-ordered input DMAs -------------------------------
            # ring A (sync): x rows 0-1 (first group's rows), then the rest
            # ring B (scalar): weight group (rp0, dh0) first
            wt_tiles = {}

            def wdma(eng, u, split_first=False):
                rp, dh = u // K, u % K
                t = wtpool.tile([128, WT_GROUP], fp16, tag="wt",
                                name=f"wt_{rp}_{dh}")
                base = u * WT_GROUP
                if split_first:
                    # per-tap chunks so compute starts immediately and is
                    # never starved inside the first group
                    for q in range(K):
                        eng.dma_start(
                            out=t[:, q * WT_TAP:(q + 1) * WT_TAP],
                            in_=wtbuf[:, base + q * WT_TAP:
                                      base + (q + 1) * WT_TAP])
                else:
                    eng.dma_start(out=t[:],
                                  in_=wtbuf[:, base:base + WT_GROUP])
                wt_tiles[(rp, dh)] = t

            # critical prefix: first tap + first x row-pair, one per ring
            wdma(nc.scalar, 0, split_first=True)     # (rp0, dh0)
            nc.sync.dma_start(out=x_t[:, 0:2 * XR], in_=xbuf[:, 0:2 * XR])
            nc.sync.dma_start(out=id_t[:], in_=ident[:])
            nc.sync.dma_start(out=x_t[:, 2 * XR:4 * XR],
                              in_=xbuf[:, 2 * XR:4 * XR])
            wdma(nc.sync, 1)                         # (rp0, dh1)
            nc.scalar.dma_start(out=x_t[:, 4 * XR:6 * XR],
                                in_=xbuf[:, 4 * XR:6 * XR])
            wdma(nc.scalar, 2)
            nc.sync.dma_start(out=x_t[:, 6 * XR:], in_=xbuf[:, 6 * XR:])
            for u in range(3, 10):
                wdma(nc.sync if u % 2 else nc.scalar, u)

            xv = x_t[:].rearrange("p (r c j) -> p r c j", r=HROWS, c=C)

            def fused_half(rp, dh, rr):
                """One DVE op for the 5 dw taps of output row (rp, dh, rr):
                [k5, c3, j516].  ACT first replicates the raw weights
                across c (5 small copies)."""
                w3 = w3pool.tile([128, W3_HALF], fp16, tag="w3",
                                 name=f"w3_{rp}_{dh}_{rr}")
                raw = wt_tiles[(rp, dh)][:].rearrange(
                    "p (k r j) -> p k r j", k=K, r=2)
                w3v = w3.rearrange("p (k c j) -> p k c j", k=K, c=C)
                for k in range(K):
                    s = raw[:, k, rr][:, None, :].broadcast_to([128, C, PJ])
                    nc.scalar.copy(out=w3v[:, k], in_=s)
                p_t = ppool.tile([128, P_HALF], fp16, tag="p",
                                 name=f"p_{rp}_{dh}_{rr}")
                pv = p_t[:].rearrange("p (k c j) -> p k c j", k=K, c=C)
                r0 = dh + 2 * rp + rr
                xs = xv[:, r0, :, 0:PJ][:, None, :, :].broadcast_to(
                    [128, K, C, PJ])
                nc.vector.tensor_mul(out=pv, in0=xs, in1=w3v)
                return pv

            def single_units(rp, dh):
                """5 single-tap DVE ops for (rp, dh) (used for the last
                group so tail matmuls interleave with production)."""
                raw = wt_tiles[(rp, dh)][:].rearrange(
                    "p (k r j) -> p k r j", k=K, r=2)
                pvs = []
                r0 = dh + 2 * rp
                xs = xv[:, r0:r0 + 2, :, 0:PJ].transpose([0, 2, 1, 3])
                for dw in range(K):
                    p_t = spool.tile([128, P_TAP], fp16, tag="s",
                                     name=f"s_{rp}_{dh}_{dw}")
                    pv = p_t[:].rearrange("p (c r j) -> p c r j", c=C, r=2)
                    wk = raw[:, dw][:, None, :, :].broadcast_to([128, C, 2, PJ])
                    nc.vector.tensor_mul(out=pv, in0=xs, in1=wk)
                    pvs.append(pv)
                return pvs

            for rp in range(2):
                ps_t = psumpool.tile([128, 6 * 512], f32, tag="ps",
                                     name=f"ps_{rp}")
                for di, dh in enumerate(range(K)):
                    first_dh = di == 0
                    last_dh = di == K - 1
                    if FUSE and not last_dh:
                        halves = [fused_half(rp, dh, rr) for rr in range(2)]
                        # mov(dw, c, rr) = halves[rr][:, dw, c, dw:dw+W]
                        movf = lambda dw, c, rr: halves[rr][:, dw, c,
                                                            dw:dw + W]
                    else:
                        singles = single_units(rp, dh)   # [p, c, r, j] each
                        movf = lambda dw, c, rr: singles[dw][:, c, rr,
                                                             dw:dw + W]
                    for dw in range(K):
                        for c in range(C):
                            for rr in range(2):
                                bank = c * 2 + rr
                                nc.tensor.matmul(
                                    ps_t[:, bank * 512:(bank + 1) * 512],
                                    id_t[:],
                                    movf(dw, c, rr),
                                    start=(first_dh and dw == 0),
                                    stop=(last_dh and dw == K - 1),
                                )

                # evacuate: ACT takes banks 0-3, DVE (idle now) banks 4-5;
                # ship each piece as soon as it lands
                o_t = outpool.tile([128, O_FREE], fp16)
                nc.scalar.copy(out=o_t[:, 0:3 * 512], in_=ps_t[:, 0:3 * 512])
                nc.vector.tensor_copy(out=o_t[:, 3 * 512:],
                                      in_=ps_t[:, 3 * 512:])
                nc.sync.dma_start(out=out[rp, :, 0:3 * 512],
                                  in_=o_t[:, 0:3 * 512])
                nc.sync.dma_start(out=out[rp, :, 3 * 512:],
                                  in_=o_t[:, 3 * 512:])

    _dedupe_identity_ldweights(nc)
    _split_multi_waits(nc)
    _NC_CACHE = nc
    return nc


def _pack_inputs(input_data: np.ndarray, weights: np.ndarray):
    """Host-side layout + fp16 conversion into per-core SBUF-ready buffers."""
    xh = input_data.astype(np.float16)     # (N, H, W, C)
    wh = weights.astype(np.float16)        # (N, H, W, KK)

    identity = np.eye(128, dtype=np.float16)
    in_maps = []
    for n in range(N_CORES):
        # x: [p, rr, c, j]: image row 4p+rr-2, col j-2 (zero pad)
        canvas = np.zeros((C, H + 4, JW), dtype=np.float16)
        canvas[:, 2:2 + H, 2:2 + W] = xh[n].transpose(2, 0, 1)
        sw = np.lib.stride_tricks.sliding_window_view(canvas, HROWS, axis=1)
        sw = sw[:, ::RPP][:, :128]            # (C, 128, JW, 8)
        X = np.ascontiguousarray(
            sw.transpose(1, 3, 0, 2).reshape(128, X_FREE))

        # weight chunks: wtbuf[p, rp, k, rr, j'] = wt[4p+2rp+rr, j'-dw, k]
        wtpad = np.zeros((H, PJ, KK), dtype=np.float16)
        for dw in range(K):
            wtpad[:, dw:dw + W, dw::K] = wh[n][:, :, dw::K]
        WT = np.ascontiguousarray(
            wtpad.reshape(128, 2, 2, PJ, KK)       # p, rp, rr, j', k
            .transpose(0, 1, 4, 2, 3)              # p, rp, k, rr, j'
            .reshape(128, WT_FREE))
        in_maps.append({"xbuf": X, "wtbuf": WT, "ident": identity})
    return in_maps


def _unpack_outputs(results) -> np.ndarray:
    out = np.empty((N, H, W, C), dtype=np.float32)
    for n in range(N_CORES):
        o = results[n]["out"].astype(np.float32)   # (2, 128, O_FREE)
        o = o.reshape(2, 128, C, 2, W)             # rp, p, c, rr, w
        # h = 4p + 2rp + rr
        out[n] = o.transpose(1, 0, 3, 4, 2).reshape(H, W, C)
    return out


def kernel(input_data: np.ndarray, weights: np.ndarray) -> np.ndarray:
    input_data = np.asarray(input_data, dtype=np.float32)
    weights = np.asarray(weights, dtype=np.float32)
    nc = _build_program()
    in_maps = _pack_inputs(input_data, weights)
    res = run_bass_kernel_spmd(nc, in_maps, list(range(N_CORES)))
    return _unpack_outputs(res.results)


if __name__ == "__main__":
    rng = np.random.default_rng(0)
    x = rng.standard_normal((N, H, W, C), dtype=np.float32)
    w = rng.standard_normal((N, H, W, KK), dtype=np.float32) * 0.1
    out = kernel(input_data=x, weights=w)

    xp = np.pad(x, ((0, 0), (2, 2), (2, 2), (0, 0)))
    exp = np.zeros_like(x)
    for k in range(KK):
        dh, dw = k // K, k % K
        exp += xp[:, dh:dh + H, dw:dw + W, :] * w[..., k:k + 1]
    diff = np.linalg.norm(out - exp) / np.linalg.norm(exp)
    print("out", out.shape, out.dtype, "rel err", diff)
